# revision 47
# baseline (speedup 1.0000x reference)
"""ANI-style AEVComputer on 8 TRN2 NeuronCores (Bass/Tile).

Strategy
--------
Radial: data-parallel, core k owns conformations (2k, 2k+1); joint
(2 conf x 64 atom) d^2 matrix via the TensorE matmul trick, gaussians,
species scatter by one-hot matmul.

Angular: triples (center i, j<k within angular cutoff) are balanced
GLOBALLY: centers (conf, atom) are bin-packed onto cores (<=128 centers
per core, triple counts equalized), so every core gets ~1/8 of all
triples (NCH chunks of 128). The host ships gathered coordinates
[Ri|Ri|Rj|Rk] per triple plus (bin, species-pair) ids; the device
computes 32 angular basis values per triple and scatters them to
(bin, species-pair) with a 128-wide one-hot matmul per chunk,
PSUM-accumulated, then DMAs PSUM straight to DRAM (f32); the host
permutes bins back to (conf, atom) rows.

All scalar-engine activations (Ln / Exp / Square) live in ONE
activation table (natural_log_exp_and_others), so there are no
mid-kernel ACT_TABLE_LOADs: sqrt(x) = exp(0.5*ln(x)), and the cosine
cutoff fc(d) = 0.5+0.5*cos(pi*d/rc) is evaluated as a degree-5
polynomial in d^2 on the vector/gpsimd engines (max err ~1e-6).
"""
import sys

if '/opt/trn_rl_repo' not in sys.path:
    sys.path.insert(0, '/opt/trn_rl_repo')

import numpy as np
import ml_dtypes

import concourse.bass as bass
import concourse.tile as tile
from concourse import mybir
from concourse.bass_utils import run_bass_kernel_spmd

DT = mybir.dt
AF = mybir.ActivationFunctionType
ALU = mybir.AluOpType

# ---------------- walrus compat: one sync wait per instruction ----------------


def _split_multiwaits(nc):
    n = 0
    for f in nc.m.functions:
        for bb in f.blocks:
            insts = bb.instructions
            out = []
            changed = False
            for inst in insts:
                si = inst.sync_info
                waits = list(si.on_wait) if si is not None else []
                if len(waits) > 1:
                    changed = True
                    for w in waits[:-1]:
                        n += 1
                        out.append(mybir.InstNoOp(
                            name=f"mwsplit-{n}", engine=inst.engine, ins=[], outs=[],
                            sync_info=mybir.SyncInfo(on_wait=[w], on_update=[]),
                        ))
                    inst.sync_info = mybir.SyncInfo(
                        on_wait=[waits[-1]], on_update=list(si.on_update))
                out.append(inst)
            if changed:
                insts.clear()
                insts.extend(out)
    return n


def _install_drain_patch():
    from concourse.tile import TileContext
    from concourse.vector_clock import ScopedClock

    def _patched(self, tick_clock, wait_clock):
        nc = self.nc
        drain_inst = nc.sync.drain()
        wait_clock.add_sem_waits(
            drain_inst.ins, ScopedClock({None: tick_clock.global_clock}))
        si = drain_inst.ins.sync_info
        waits = list(si.on_wait) if si else []
        if len(waits) > 1:
            drain_inst.ins.sync_info = mybir.SyncInfo(
                on_wait=waits[:1], on_update=[])
            engs = [nc.vector, nc.scalar, nc.gpsimd, nc.tensor, nc.sync]
            for idx, wt in enumerate(waits[1:]):
                e = engs[idx % len(engs)]
                nop = e.nop(nofuse=True)
                nop.ins.sync_info = mybir.SyncInfo(on_wait=[wt], on_update=[])
        nc.all_engine_barrier()
        assert self.sems is not None
        popped = nc._tile_sem_poison_stack.pop()
        assert popped is self._sem_poison
        nc.clear_and_free_semaphores(list(self.sems.allocated().values()))
        nc.all_engine_barrier()

    TileContext._drain_and_barrier = _patched


_install_drain_patch()

# ---------------- problem constants ----------------
RCR, RCA = 5.2, 3.5
SHF_R = (0.9 + 0.26875 * np.arange(16)).astype(np.float64)
SHF_A = np.array([0.9, 1.55, 2.2, 2.85], np.float64)
SHF_Z = (np.pi / 16 + (np.pi / 8) * np.arange(8)).astype(np.float64)
NSP = 4
C, A = 16, 64
A2 = 2 * A
NCORES, CPC = 8, 2

_tbl = np.zeros((NSP, NSP), np.int64)
_k = 0
for _a in range(NSP):
    for _b in range(_a, NSP):
        _tbl[_a, _b] = _tbl[_b, _a] = _k
        _k += 1
NPAIR_T = _k                  # 10


def _fit_fc_poly(rc, scale, deg=5):
    """scale*(0.5+0.5*cos(pi*sqrt(x)/rc)) as poly in x=d^2 over [0, rc^2]."""
    x = np.linspace(0, rc * rc, 8001)
    y = scale * (0.5 + 0.5 * np.cos(np.pi * np.sqrt(x) / rc))
    c = np.polynomial.chebyshev.Chebyshev.fit(x, y, deg, domain=[0, rc * rc])
    return c.convert(kind=np.polynomial.Polynomial).coef  # a0..a5


PC_R = _fit_fc_poly(RCR, 0.25)   # radial fc includes the 0.25 prefactor
PC_A = _fit_fc_poly(RCA, 1.0)

# lane-constant table columns
_LN_SHFA2 = 0                 # 4: 2*sqrt(2)*SHF_A
_LN_CZ = 4                    # 8: 0.475*cos(SHF_Z)
_LN_SZ = 12                   # 8: 0.5*sin(SHF_Z)
_LN_LN2 = 20                  # 1: ln(2)
_LN_HALF = 21                 # 1: 0.5
_LN_N = 22

_NC_CACHE = {}


def _poly5(nc, eng, out_view, x, coefs, msk, tmp):
    """out_view = poly5(x) * msk.

    On DVE: 1 tensor_scalar + 5 scalar_tensor_tensor (STT illegal on Pool):
    t <- x*a5 ; t <- (t+a4)*x ; ... ; out = (t+a0)*msk
    On Pool: tensor_scalar/tensor_tensor pairs (10 ops).
    """
    a = [float(v) for v in coefs]
    eng.tensor_scalar(tmp, x, a[5], None, ALU.mult)
    if eng is nc.vector:
        for k in (4, 3, 2, 1):
            eng.scalar_tensor_tensor(tmp, tmp, a[k], x, ALU.add, ALU.mult)
        eng.scalar_tensor_tensor(out_view, tmp, a[0], msk, ALU.add, ALU.mult)
    else:
        for k in (4, 3, 2, 1):
            eng.tensor_scalar(tmp, tmp, a[k], None, ALU.add)
            eng.tensor_tensor(tmp, tmp, x, ALU.mult)
        eng.tensor_scalar(tmp, tmp, a[0], None, ALU.add)
        eng.tensor_tensor(out_view, tmp, msk, ALU.mult)


def _build(NCH):
    """Per-core Bass graph; NCH = number of 128-triple chunks."""
    nc = bass.Bass("TRN2", target_bir_lowering=False, debug=False)

    W_RJK = 12 * NCH
    W_LN = W_RJK
    WCOMBO = W_LN + _LN_N

    coords = nc.declare_dram_parameter("coords", [35, A], DT.float32, isOutput=False)
    combo = nc.declare_dram_parameter("combo", [A2, WCOMBO], DT.float32, isOutput=False)
    # host-built one-hots: [species-onehot (8) | pair-onehot expanded to
    # 320 cols per chunk (NCH*320)] bf16
    ohb = nc.declare_dram_parameter("ohb", [A2, 2 * NSP + NCH * NPAIR_T * 32],
                                    DT.bfloat16, isOutput=False)
    ohci_d = nc.declare_dram_parameter("ohci", [A2, NCH * 128], DT.bfloat16,
                                       isOutput=False)
    # radial out ships in the on-chip [species-block, atom, shift] layout;
    # the host permutes (pure indexing) - keeps the DMA fully contiguous
    out = nc.declare_dram_parameter("out", [2 * NSP, 16, A], DT.float32, isOutput=True)
    outa = nc.declare_dram_parameter("outa", [A2, NPAIR_T * 32], DT.float32, isOutput=True)

    with tile.TileContext(nc) as tc:
        with tc.tile_pool(name="sb", bufs=1) as sb, \
             tc.tile_pool(name="ps", bufs=1, space="PSUM") as ps:

            # ---- engine warmups (absorb first-op overhead during DMA) ----
            wrm = sb.tile([4, 4], DT.float32)
            nc.gpsimd.memset(wrm[:], 1.0)
            nc.vector.tensor_scalar(wrm[:], wrm[:], 1.0, None, ALU.mult)
            nc.scalar.copy(wrm[:], wrm[:])
            wps = ps.tile([4, 4], DT.float32, tag="wps")
            nc.tensor.matmul(wps[:], wrm[:], wrm[:], start=True, stop=True)

            # ---- input DMAs: critical-first, the big one-hot block last ----
            # conf cc's 3 coordinate rows live at partition base 32*cc
            # (matmul stationary/moving require base partition 0/32/64);
            # host ships the padded [35, A] layout so this is ONE dma.
            combo_sb = sb.tile([A2, WCOMBO], DT.float32)
            nc.sync.dma_start(combo_sb[:], combo[:])
            ct2 = sb.tile([35, A], DT.float32)
            nc.sync.dma_start(ct2[:], coords[:])
            ohci = sb.tile([A2, NCH, 128], DT.bfloat16)
            nc.sync.dma_start(ohci[:], ohci_d[:].rearrange(
                "p (c m) -> p c m", m=128))

            rjk = combo_sb[:, 0:W_RJK].rearrange("p (c k) -> p c k", k=12)
            lanes_sb = combo_sb[:, W_LN:W_LN + _LN_N]

            def lane(c0, n, w):
                return lanes_sb[:, c0:c0 + n].rearrange(
                    "p (c k) -> p c k", c=1).broadcast_to([A2, w, n])

            # radial shift constants + the big one-hot DMA issued LAST on
            # gpsimd so its 0.9MB doesn't block the critical input queues
            ones3 = sb.tile([35, A], DT.float32)
            nc.gpsimd.memset(ones3[:], 1.0)
            shfrq = sb.tile([A2, 16, A], DT.float32)
            for rr in range(16):
                nc.gpsimd.memset(shfrq[:, rr, :], 4.0 * SHF_R[rr])
            ohb_sb = sb.tile([A2, 2 * NSP + NCH * NPAIR_T * 32], DT.bfloat16)
            nc.gpsimd.dma_start(ohb_sb[:], ohb[:])
            ohs_sb = ohb_sb[:, 0:2 * NSP]
            ohsp320 = ohb_sb[:, 2 * NSP:].rearrange(
                "p (c m) -> p c m", m=NPAIR_T * 32)

            # ---------- radial d^2 front (joint 2 confs; no scalar engine:
            # the initial ACT_TABLE_LOAD would stall it) ----------
            NSQ = A + 3 * NCH
            sqin = sb.tile([A2, NSQ], DT.float32)
            sq3 = sb.tile([35, A], DT.float32)
            m2ct = sb.tile([35, A], DT.float32)
            for cc in range(CPC):
                c0 = 32 * cc
                nc.vector.tensor_tensor(sq3[c0:c0 + 3, :], ct2[c0:c0 + 3, :],
                                        ct2[c0:c0 + 3, :], ALU.mult)
                nc.vector.tensor_scalar(m2ct[c0:c0 + 3, :], ct2[c0:c0 + 3, :],
                                        -2.0, None, ALU.mult)
            dsqp = []
            for cc in range(CPC):
                dp = ps.tile([A, A], DT.float32, tag=f"dsq{cc}", name=f"dsq{cc}")
                c0 = 32 * cc
                nc.tensor.matmul(dp[:], sq3[c0:c0 + 3], ones3[c0:c0 + 3],
                                 start=True, stop=False)
                nc.tensor.matmul(dp[:], ones3[c0:c0 + 3], sq3[c0:c0 + 3],
                                 start=False, stop=False)
                nc.tensor.matmul(dp[:], ct2[c0:c0 + 3], m2ct[c0:c0 + 3],
                                 start=False, stop=True)
                dsqp.append(dp)
            for cc in range(CPC):
                nc.vector.tensor_scalar(sqin[A * cc:A * (cc + 1), 0:A], dsqp[cc][:],
                                        1e-12, None, ALU.max)

            # ---------- triple stream geometry (vector-only chain -> Ln) ----
            v12 = sb.tile([A2, NCH, 2, 3], DT.float32)
            nc.vector.tensor_tensor(
                v12[:].rearrange("p c u x -> p c (u x)"),
                rjk[:, :, 0:6], rjk[:, :, 6:12], ALU.subtract)
            sq6 = sb.tile([A2, NCH, 2, 3], DT.float32)
            nc.vector.tensor_tensor(sq6[:], v12[:], v12[:], ALU.mult)
            dq2 = sqin[:, A:A + 2 * NCH].rearrange("p (c u) -> p c u", u=2)
            nc.vector.tensor_reduce(dq2, sq6[:], mybir.AxisListType.X, ALU.add)
            dm3 = sb.tile([A2, NCH, 3], DT.float32)
            nc.vector.tensor_tensor(dm3[:], v12[:, :, 0, :], v12[:, :, 1, :], ALU.mult)
            dot = sb.tile([A2, NCH, 1], DT.float32)
            nc.vector.tensor_reduce(dot[:], dm3[:], mybir.AxisListType.X, ALU.add)
            dqp = sb.tile([A2, NCH, 1], DT.float32)
            nc.vector.tensor_tensor(dqp[:], dq2[:, :, 0:1], dq2[:, :, 1:2], ALU.mult)
            dot2 = sb.tile([A2, NCH, 1], DT.float32)
            nc.vector.tensor_tensor(dot2[:], dot[:], dot[:], ALU.mult)
            rcpq = sb.tile([A2, NCH, 1], DT.float32)
            nc.vector.reciprocal(rcpq[:], dqp[:])
            usq = sb.tile([A2, NCH, 1], DT.float32)
            nc.vector.tensor_tensor(usq[:], dot2[:], rcpq[:], ALU.mult)
            nc.vector.tensor_scalar(
                sqin[:, A + 2 * NCH:NSQ].rearrange("p (c u) -> p c u", u=1),
                usq[:], -0.9025, 1.0, ALU.mult, ALU.add)

            # ---------- Ln + Exp: d = exp(0.5*ln(d^2)), ss likewise.
            # Pair part first (feeds the angular chain), radial part second.
            lnall = sb.tile([A2, NSQ], DT.float32)
            dall = sb.tile([A2, NSQ], DT.float32)
            nc.scalar.activation(lnall[:, A:NSQ], sqin[:, A:NSQ], AF.Ln)
            nc.scalar.activation(dall[:, A:NSQ], lnall[:, A:NSQ], AF.Exp, scale=0.5)
            nc.scalar.activation(lnall[:, 0:A], sqin[:, 0:A], AF.Ln)
            nc.scalar.activation(dall[:, 0:A], lnall[:, 0:A], AF.Exp, scale=0.5)
            d_t = dall[:, 0:A]
            d2l = dall[:, A:A + 2 * NCH].rearrange("p (c u) -> p c u", u=2)
            ss = dall[:, A + 2 * NCH:NSQ].rearrange("p (c u) -> p c u", u=1)

            # ---------- cutoff polynomials in d^2 (vector; overlap Ln/Exp) ----
            msk2 = sb.tile([A2, NCH, 2], DT.float32)
            d2p = sqin[:, A:A + 2 * NCH].rearrange("p (c u) -> p c u", u=2)
            nc.vector.tensor_scalar(msk2[:], d2p, RCA * RCA, None, ALU.is_le)
            g2 = sb.tile([A2, NCH, 2], DT.float32)
            tmpA = sb.tile([A2, NCH, 2], DT.float32)
            _poly5(nc, nc.vector, g2[:], d2p, PC_A, msk2[:], tmpA[:])
            g = sb.tile([A2, NCH, 1], DT.float32)
            nc.gpsimd.tensor_tensor(g[:], g2[:, :, 0:1], g2[:, :, 1:2], ALU.mult)

            mskR = sb.tile([A2, A], DT.float32)
            nc.vector.tensor_scalar(mskR[:], sqin[:, 0:A], RCR * RCR, None, ALU.is_le)
            fcR = sb.tile([A2, A], DT.float32)
            tmpR = sb.tile([A2, A], DT.float32)
            _poly5(nc, nc.vector, fcR[:], sqin[:, 0:A], PC_R, mskR[:], tmpR[:])

            # ---------- post-d angular chain (vector; gp only for leaves) ----
            prod = sb.tile([A2, NCH, 1], DT.float32)
            nc.gpsimd.tensor_tensor(prod[:], d2l[:, :, 0:1], d2l[:, :, 1:2], ALU.mult)
            tsum = sb.tile([A2, NCH, 1], DT.float32)
            nc.gpsimd.tensor_tensor(tsum[:], d2l[:, :, 0:1], d2l[:, :, 1:2], ALU.add)
            rcp = sb.tile([A2, NCH, 1], DT.float32)
            nc.vector.reciprocal(rcp[:], prod[:])
            u = sb.tile([A2, NCH, 1], DT.float32)
            nc.vector.tensor_tensor(u[:], dot[:], rcp[:], ALU.mult)

            am = sb.tile([A2, NCH, 4], DT.float32)
            nc.vector.scalar_tensor_tensor(
                am[:], tsum[:].broadcast_to([A2, NCH, 4]), float(np.sqrt(2.0)),
                lane(_LN_SHFA2, 4, NCH), ALU.mult, ALU.subtract)
            amsq = sb.tile([A2, NCH, 4], DT.float32)
            nc.gpsimd.tensor_tensor(amsq[:], am[:], am[:], ALU.mult)
            f2 = sb.tile([A2, NCH, 4], DT.float32)
            nc.scalar.activation(f2[:], amsq[:], AF.Exp, scale=-1.0)

            # hcl = hc + hs; the +0.5 rides the Ln bias. 0.5(1+cos(theta-shz))
            # >= 0.065 for all reachable angles, so no clamp is needed.
            hc = sb.tile([A2, NCH, 8], DT.float32)
            hs = sb.tile([A2, NCH, 8], DT.float32)
            # hcl and d4 share one tile: the tile-granular dependency tracker
            # then orders the radial rsub AFTER the f1 chain's hcl, keeping
            # the latency-critical f1 path ahead of radial bulk work
            hclx = sb.tile([A2, NCH * 8 + A], DT.float32)
            hcl = hclx[:, 0:NCH * 8].rearrange("p (c z) -> p c z", z=8)
            d4 = hclx[:, NCH * 8:NCH * 8 + A]
            nc.vector.tensor_tensor(hc[:], u[:].broadcast_to([A2, NCH, 8]),
                                    lane(_LN_CZ, 8, NCH), ALU.mult)
            nc.vector.tensor_tensor(hs[:], ss.broadcast_to([A2, NCH, 8]),
                                    lane(_LN_SZ, 8, NCH), ALU.mult)
            nc.vector.tensor_tensor(hcl, hc[:], hs[:], ALU.add)
            lnh = sb.tile([A2, NCH, 8], DT.float32)
            f1 = sb.tile([A2, NCH, 8], DT.bfloat16)
            nc.scalar.activation(lnh[:], hcl, AF.Ln,
                                 bias=lanes_sb[:, _LN_HALF:_LN_HALF + 1])
            nc.scalar.activation(f1[:], lnh[:], AF.Exp, scale=32.0,
                                 bias=lanes_sb[:, _LN_LN2:_LN_LN2 + 1])

            # radial argument on vector halves while scalar runs Ln/f1
            nc.vector.tensor_scalar(d4, d_t, 4.0, None, ALU.mult)
            rsub = sb.tile([A2, 16, A], DT.float32)
            d4b = d4.rearrange("p (r i) -> p r i", r=1)
            nc.vector.tensor_tensor(rsub[:, 0:8], d4b.broadcast_to([A2, 8, A]),
                                    shfrq[:, 0:8], ALU.subtract)
            nc.vector.tensor_tensor(rsub[:, 8:16], d4b.broadcast_to([A2, 8, A]),
                                    shfrq[:, 8:16], ALU.subtract)

            f2g = sb.tile([A2, NCH, 4], DT.bfloat16)
            nc.vector.tensor_tensor(f2g[:], f2[:], g[:].broadcast_to([A2, NCH, 4]),
                                    ALU.mult)
            at = sb.tile([A2, NCH, 32], DT.bfloat16)
            nc.vector.tensor_tensor(
                at[:].rearrange("p c (a z) -> p c a z", a=4),
                f1[:].rearrange("p c (a z) -> p c a z", a=1
                                ).broadcast_to([A2, NCH, 4, 8]),
                f2g[:].rearrange("p c (a z) -> p c a z", z=1
                                 ).broadcast_to([A2, NCH, 4, 8]),
                ALU.mult)

            # species-pair expansion with host-materialized one-hot, all on
            # vector (concurrent gpsimd wide ops just steal DVE bandwidth);
            # 4 slices so the matmuls chase, radial rtf interleaved
            at320 = sb.tile([A2, NCH, NPAIR_T, 32], DT.bfloat16)
            angp = ps.tile([A2, NPAIR_T * 32], DT.float32, tag="angp")

            def build320(c0, c1):
                nc.vector.tensor_tensor(
                    at320[:, c0:c1],
                    at[:, c0:c1].rearrange("p c (s w) -> p c s w", s=1
                                           ).broadcast_to([A2, c1 - c0, NPAIR_T, 32]),
                    ohsp320[:, c0:c1].rearrange("p c (s w) -> p c s w", s=NPAIR_T),
                    ALU.mult)

            # radial: squares + exps on scalar halves, fc multiply on vector
            # interleaved with the build320 slices; radsb keeps the PSUM
            # [species, shift, atom] layout (host permutes)
            rsq = sb.tile([A2, 16, A], DT.float32)
            rte = sb.tile([A2, 16, A], DT.float32)
            rtf = sb.tile([A2, 16, A], DT.bfloat16)
            fcRb = fcR[:].rearrange("p (r i) -> p r i", r=1)
            radsb = sb.tile([2 * NSP, 16, A], DT.float32)
            for half in range(2):
                h0 = 8 * half
                nc.scalar.square(rsq[:, h0:h0 + 8], rsub[:, h0:h0 + 8])
                nc.scalar.activation(rte[:, h0:h0 + 8], rsq[:, h0:h0 + 8],
                                     AF.Exp, scale=-1.0)

            q4 = max(1, NCH // 4)
            cuts = [0, q4, 2 * q4, 3 * q4, NCH]
            radp = []
            for s in range(4):
                if cuts[s + 1] > cuts[s]:
                    build320(cuts[s], cuts[s + 1])
                for ch in range(cuts[s], cuts[s + 1]):
                    nc.tensor.matmul(angp[:], ohci[:, ch, :], at320[:, ch],
                                     start=(ch == 0), stop=(ch == NCH - 1))
                if s in (1, 2):
                    half = s - 1
                    h0 = 8 * half
                    nc.vector.tensor_tensor(rtf[:, h0:h0 + 8], rte[:, h0:h0 + 8],
                                            fcRb.broadcast_to([A2, 8, A]), ALU.mult)
                    rp = ps.tile([2 * NSP, 8, A], DT.float32, tag="radp",
                                 name=f"radp{half}")
                    nc.tensor.matmul(rp[:], ohs_sb[:], rtf[:, h0:h0 + 8, :],
                                     start=True, stop=True)
                    radp.append(rp)
                    nc.scalar.copy(radsb[:, h0:h0 + 8, :], rp[:])
                    nc.gpsimd.dma_start(out[:, h0:h0 + 8, :],
                                        radsb[:, h0:h0 + 8, :])

            # angular: PSUM -> SBUF (split; gpsimd can't read PSUM), DMA
            # halves pipelined behind the copies
            angsb = sb.tile([A2, NPAIR_T * 32], DT.float32)
            nc.scalar.copy(angsb[:, 0:112], angp[:, 0:112])
            nc.sync.dma_start(outa[:, 0:112], angsb[:, 0:112])
            nc.vector.tensor_copy(angsb[:, 112:320], angp[:, 112:320])
            nc.sync.dma_start(outa[:, 112:320], angsb[:, 112:320])

    _split_multiwaits(nc)
    return nc


# ---------------- host side ----------------

def _prep(species, coordinates):
    sp = np.clip(np.asarray(species).astype(np.int64), 0, NSP - 1)
    co = np.ascontiguousarray(np.asarray(coordinates), dtype=np.float32)
    d2 = ((co[:, :, None, :].astype(np.float64) - co[:, None, :, :]) ** 2).sum(-1)
    D = np.sqrt(d2)
    for c in range(C):
        np.fill_diagonal(D[c], 1e9)
    near = D < (RCA + 0.02)

    # triples grouped per center (c, i)
    centers = []  # (count, c, i, J, K)
    for c in range(C):
        for i in range(A):
            nz = np.nonzero(near[c, i])[0]
            m = nz.size
            if m >= 2:
                jj, kk = np.triu_indices(m, k=1)
                centers.append((jj.size, c, i, nz[jj], nz[kk]))
    centers.sort(key=lambda t: -t[0])

    # greedy bin-pack: <=128 centers/core, balance triple counts
    loads = [0] * NCORES
    nbins = [0] * NCORES
    assign = [[] for _ in range(NCORES)]
    for ent in centers:
        k = min((kk for kk in range(NCORES) if nbins[kk] < A2),
                key=lambda kk: loads[kk])
        assign[k].append(ent)
        loads[k] += ent[0]
        nbins[k] += 1

    NCH = max(1, int(np.ceil(max(loads) / 128)))
    KT = NCH * 128

    lane_row = np.zeros(_LN_N, np.float64)
    lane_row[_LN_SHFA2:_LN_SHFA2 + 4] = 2.0 * np.sqrt(2.0) * SHF_A
    lane_row[_LN_CZ:_LN_CZ + 8] = 0.475 * np.cos(SHF_Z)
    lane_row[_LN_SZ:_LN_SZ + 8] = 0.5 * np.sin(SHF_Z)
    lane_row[_LN_LN2] = np.log(2.0)
    lane_row[_LN_HALF] = 0.5

    pad_rjk = np.array([0, 0, 0, 0, 0, 0, 60, 0, 0, 0, 60, 0], np.float32)

    per_core = []
    bin_maps = []
    for k in range(NCORES):
        rjk_l, ci_l, spid_l = [], [], []
        bmap = []
        for b, (cnt, c, i, J, K) in enumerate(assign[k]):
            bmap.append((c, i))
            rjk_l.append(np.concatenate(
                [np.repeat(co[c, i][None], cnt, 0),
                 np.repeat(co[c, i][None], cnt, 0),
                 co[c, J], co[c, K]], axis=1))
            ci_l.append(np.full(cnt, b, np.float32))
            spid_l.append(_tbl[sp[c, J], sp[c, K]].astype(np.float32))
        bin_maps.append(bmap)
        T = sum(x.size for x in ci_l)
        rjk_f = np.empty((KT, 12), np.float32)
        rjk_f[:] = pad_rjk
        ci_f = np.zeros(KT, np.float32)
        spid_f = np.zeros(KT, np.float32)
        if T:
            rjk_f[:T] = np.concatenate(rjk_l, axis=0)
            ci_f[:T] = np.concatenate(ci_l)
            spid_f[:T] = np.concatenate(spid_l)
        # triple t = ch*128 + p  ->  tile [p, ch]
        rjk_t = rjk_f.reshape(NCH, 128, 12).transpose(1, 0, 2).reshape(128, -1)
        ci_t = ci_f.reshape(NCH, 128).T
        spid_t = spid_f.reshape(NCH, 128).T
        lanes_t = np.tile(lane_row.astype(np.float32), (A2, 1))
        combo = np.concatenate([rjk_t, lanes_t], axis=1)

        # host-built one-hots (bf16 exact for 0/1); pair one-hot expanded
        # to the full 320 columns per chunk
        ohci = (ci_t[:, :, None] == np.arange(128, dtype=np.float32)
                ).astype(ml_dtypes.bfloat16).reshape(128, -1)
        ohsp320 = np.repeat(
            (spid_t[:, :, None] == np.arange(NPAIR_T, dtype=np.float32)),
            32, axis=2).astype(ml_dtypes.bfloat16).reshape(128, -1)

        ca, cb = 2 * k, 2 * k + 1
        ohsv = np.zeros((A2, 2 * NSP), np.float32)
        for cc, c in enumerate((ca, cb)):
            ohsv[A * cc:A * (cc + 1), NSP * cc:NSP * (cc + 1)] = (
                sp[c][:, None] == np.arange(NSP))
        ohb = np.concatenate([ohsv.astype(ml_dtypes.bfloat16), ohsp320], axis=1)
        cpad = np.zeros((35, A), np.float32)
        cpad[0:3] = co[ca].T
        cpad[32:35] = co[cb].T
        per_core.append({
            "coords": cpad,
            "combo": np.ascontiguousarray(combo),
            "ohb": np.ascontiguousarray(ohb),
            "ohci": np.ascontiguousarray(ohci),
        })
    return NCH, per_core, bin_maps


def _run(species, coordinates, trace=False):
    NCH, in_maps, bin_maps = _prep(species, coordinates)
    if NCH not in _NC_CACHE:
        _NC_CACHE[NCH] = _build(NCH)
    nc = _NC_CACHE[NCH]
    res = run_bass_kernel_spmd(nc, in_maps, core_ids=list(range(NCORES)), trace=trace)
    full = np.zeros((C, A, 384), np.float32)
    for k in range(NCORES):
        orad = np.asarray(res.results[k]["out"])   # (2*NSP, 16, A)
        oang = np.asarray(res.results[k]["outa"])
        for cc in range(CPC):
            full[2 * k + cc, :, 0:64] = (
                orad[NSP * cc:NSP * (cc + 1)].transpose(2, 0, 1).reshape(A, 64))
        for b, (c, i) in enumerate(bin_maps[k]):
            full[c, i, 64:] = oang[b]
    return full, res


def kernel(species, coordinates):
    out, _ = _run(species, coordinates, trace=False)
    return out


# revision 50
# speedup vs baseline: 1.0402x; 1.0402x over previous
"""ANI-style AEVComputer on 8 TRN2 NeuronCores (Bass/Tile).

Strategy
--------
Radial: data-parallel, core k owns conformations (2k, 2k+1); joint
(2 conf x 64 atom) d^2 matrix via the TensorE matmul trick, gaussians,
species scatter by one-hot matmul.

Angular: triples (center i, j<k within angular cutoff) are balanced
GLOBALLY: centers (conf, atom) are bin-packed onto cores (<=128 centers
per core, triple counts equalized), so every core gets ~1/8 of all
triples (NCH chunks of 128). The host ships gathered coordinates
[Ri|Ri|Rj|Rk] per triple plus (bin, species-pair) ids; the device
computes 32 angular basis values per triple and scatters them to
(bin, species-pair) with a 128-wide one-hot matmul per chunk,
PSUM-accumulated, then DMAs PSUM straight to DRAM (f32); the host
permutes bins back to (conf, atom) rows.

All scalar-engine activations (Ln / Exp / Square) live in ONE
activation table (natural_log_exp_and_others), so there are no
mid-kernel ACT_TABLE_LOADs: sqrt(x) = exp(0.5*ln(x)), and the cosine
cutoff fc(d) = 0.5+0.5*cos(pi*d/rc) is evaluated as a degree-5
polynomial in d^2 on the vector/gpsimd engines (max err ~1e-6).
"""
import sys

if '/opt/trn_rl_repo' not in sys.path:
    sys.path.insert(0, '/opt/trn_rl_repo')

import numpy as np
import ml_dtypes

import concourse.bass as bass
import concourse.tile as tile
from concourse import mybir
from concourse.bass_utils import run_bass_kernel_spmd

DT = mybir.dt
AF = mybir.ActivationFunctionType
ALU = mybir.AluOpType

# ---------------- walrus compat: one sync wait per instruction ----------------


def _split_multiwaits(nc):
    n = 0
    for f in nc.m.functions:
        for bb in f.blocks:
            insts = bb.instructions
            out = []
            changed = False
            for inst in insts:
                si = inst.sync_info
                waits = list(si.on_wait) if si is not None else []
                if len(waits) > 1:
                    changed = True
                    for w in waits[:-1]:
                        n += 1
                        out.append(mybir.InstNoOp(
                            name=f"mwsplit-{n}", engine=inst.engine, ins=[], outs=[],
                            sync_info=mybir.SyncInfo(on_wait=[w], on_update=[]),
                        ))
                    inst.sync_info = mybir.SyncInfo(
                        on_wait=[waits[-1]], on_update=list(si.on_update))
                out.append(inst)
            if changed:
                insts.clear()
                insts.extend(out)
    return n


def _install_drain_patch():
    from concourse.tile import TileContext
    from concourse.vector_clock import ScopedClock

    def _patched(self, tick_clock, wait_clock):
        nc = self.nc
        drain_inst = nc.sync.drain()
        wait_clock.add_sem_waits(
            drain_inst.ins, ScopedClock({None: tick_clock.global_clock}))
        si = drain_inst.ins.sync_info
        waits = list(si.on_wait) if si else []
        if len(waits) > 1:
            drain_inst.ins.sync_info = mybir.SyncInfo(
                on_wait=waits[:1], on_update=[])
            engs = [nc.vector, nc.scalar, nc.gpsimd, nc.tensor, nc.sync]
            for idx, wt in enumerate(waits[1:]):
                e = engs[idx % len(engs)]
                nop = e.nop(nofuse=True)
                nop.ins.sync_info = mybir.SyncInfo(on_wait=[wt], on_update=[])
        nc.all_engine_barrier()
        assert self.sems is not None
        popped = nc._tile_sem_poison_stack.pop()
        assert popped is self._sem_poison
        nc.clear_and_free_semaphores(list(self.sems.allocated().values()))
        nc.all_engine_barrier()

    TileContext._drain_and_barrier = _patched


_install_drain_patch()

# ---------------- problem constants ----------------
RCR, RCA = 5.2, 3.5
SHF_R = (0.9 + 0.26875 * np.arange(16)).astype(np.float64)
SHF_A = np.array([0.9, 1.55, 2.2, 2.85], np.float64)
SHF_Z = (np.pi / 16 + (np.pi / 8) * np.arange(8)).astype(np.float64)
NSP = 4
C, A = 16, 64
A2 = 2 * A
NCORES, CPC = 8, 2

_tbl = np.zeros((NSP, NSP), np.int64)
_k = 0
for _a in range(NSP):
    for _b in range(_a, NSP):
        _tbl[_a, _b] = _tbl[_b, _a] = _k
        _k += 1
NPAIR_T = _k                  # 10


def _fit_fc_poly(rc, scale, deg=5):
    """scale*(0.5+0.5*cos(pi*sqrt(x)/rc)) as poly in x=d^2 over [0, rc^2]."""
    x = np.linspace(0, rc * rc, 8001)
    y = scale * (0.5 + 0.5 * np.cos(np.pi * np.sqrt(x) / rc))
    c = np.polynomial.chebyshev.Chebyshev.fit(x, y, deg, domain=[0, rc * rc])
    return c.convert(kind=np.polynomial.Polynomial).coef  # a0..a5


PC_R = _fit_fc_poly(RCR, 0.25)   # radial fc includes the 0.25 prefactor
PC_A = _fit_fc_poly(RCA, 1.0)

# lane-constant table columns
_LN_SHFA2 = 0                 # 4: 2*sqrt(2)*SHF_A
_LN_CZ = 4                    # 8: 0.475*cos(SHF_Z)
_LN_SZ = 12                   # 8: 0.5*sin(SHF_Z)
_LN_LN2 = 20                  # 1: ln(2)
_LN_HALF = 21                 # 1: 0.5
_LN_N = 22

_NC_CACHE = {}


def _poly5(nc, eng, out_view, x, coefs, msk, tmp):
    """out_view = poly5(x) * msk.

    On DVE: 1 tensor_scalar + 5 scalar_tensor_tensor (STT illegal on Pool):
    t <- x*a5 ; t <- (t+a4)*x ; ... ; out = (t+a0)*msk
    On Pool: tensor_scalar/tensor_tensor pairs (10 ops).
    """
    a = [float(v) for v in coefs]
    eng.tensor_scalar(tmp, x, a[5], None, ALU.mult)
    if eng is nc.vector:
        for k in (4, 3, 2, 1):
            eng.scalar_tensor_tensor(tmp, tmp, a[k], x, ALU.add, ALU.mult)
        eng.scalar_tensor_tensor(out_view, tmp, a[0], msk, ALU.add, ALU.mult)
    else:
        for k in (4, 3, 2, 1):
            eng.tensor_scalar(tmp, tmp, a[k], None, ALU.add)
            eng.tensor_tensor(tmp, tmp, x, ALU.mult)
        eng.tensor_scalar(tmp, tmp, a[0], None, ALU.add)
        eng.tensor_tensor(out_view, tmp, msk, ALU.mult)


def _build(NCH):
    """Per-core Bass graph; NCH = number of 128-triple chunks."""
    nc = bass.Bass("TRN2", target_bir_lowering=False, debug=False)

    W_RJK = 12 * NCH
    W_LN = W_RJK
    WCOMBO = W_LN + _LN_N

    coords = nc.declare_dram_parameter("coords", [35, A], DT.float32, isOutput=False)
    combo = nc.declare_dram_parameter("combo", [A2, WCOMBO], DT.float32, isOutput=False)
    # host-built one-hots: [species-onehot (8) | pair-onehot expanded to
    # 320 cols per chunk (NCH*320)] bf16
    ohb = nc.declare_dram_parameter("ohb", [A2, 2 * NSP + NCH * NPAIR_T * 32],
                                    DT.bfloat16, isOutput=False)
    ohci_d = nc.declare_dram_parameter("ohci", [A2, NCH * 128], DT.bfloat16,
                                       isOutput=False)
    # radial out ships in the on-chip [species-block, atom, shift] layout;
    # the host permutes (pure indexing) - keeps the DMA fully contiguous
    out = nc.declare_dram_parameter("out", [2 * NSP, 16, A], DT.float32, isOutput=True)
    outa = nc.declare_dram_parameter("outa", [A2, NPAIR_T * 32], DT.float32, isOutput=True)

    with tile.TileContext(nc) as tc:
        with tc.tile_pool(name="sb", bufs=1) as sb, \
             tc.tile_pool(name="ps", bufs=1, space="PSUM") as ps:

            # ---- engine warmups (absorb first-op overhead during DMA) ----
            wrm = sb.tile([4, 4], DT.float32)
            nc.gpsimd.memset(wrm[:], 1.0)
            nc.vector.tensor_scalar(wrm[:], wrm[:], 1.0, None, ALU.mult)
            nc.scalar.copy(wrm[:], wrm[:])
            wps = ps.tile([4, 4], DT.float32, tag="wps")
            nc.tensor.matmul(wps[:], wrm[:], wrm[:], start=True, stop=True)

            # ---- input DMAs: critical-first, the big one-hot block last ----
            # conf cc's 3 coordinate rows live at partition base 32*cc
            # (matmul stationary/moving require base partition 0/32/64);
            # host ships the padded [35, A] layout so this is ONE dma.
            combo_sb = sb.tile([A2, WCOMBO], DT.float32)
            nc.sync.dma_start(combo_sb[:], combo[:])
            ct2 = sb.tile([35, A], DT.float32)
            nc.sync.dma_start(ct2[:], coords[:])
            ohci = sb.tile([A2, NCH, 128], DT.bfloat16)
            nc.sync.dma_start(ohci[:], ohci_d[:].rearrange(
                "p (c m) -> p c m", m=128))

            rjk = combo_sb[:, 0:W_RJK].rearrange("p (c k) -> p c k", k=12)
            lanes_sb = combo_sb[:, W_LN:W_LN + _LN_N]

            def lane(c0, n, w):
                return lanes_sb[:, c0:c0 + n].rearrange(
                    "p (c k) -> p c k", c=1).broadcast_to([A2, w, n])

            # radial shift constants + the big one-hot DMA issued LAST on
            # gpsimd so its 0.9MB doesn't block the critical input queues
            ones3 = sb.tile([35, A], DT.float32)
            nc.gpsimd.memset(ones3[:], 1.0)
            shfrq = sb.tile([A2, 16, A], DT.float32)
            for rr in range(16):
                nc.gpsimd.memset(shfrq[:, rr, :], 4.0 * SHF_R[rr])
            ohb_sb = sb.tile([A2, 2 * NSP + NCH * NPAIR_T * 32], DT.bfloat16)
            nc.gpsimd.dma_start(ohb_sb[:], ohb[:])
            ohs_sb = ohb_sb[:, 0:2 * NSP]
            ohsp320 = ohb_sb[:, 2 * NSP:].rearrange(
                "p (c m) -> p c m", m=NPAIR_T * 32)

            # ---------- radial d^2 front (joint 2 confs; no scalar engine:
            # the initial ACT_TABLE_LOAD would stall it) ----------
            NSQ = A + 3 * NCH
            sqin = sb.tile([A2, NSQ], DT.float32)
            sq3 = sb.tile([35, A], DT.float32)
            m2ct = sb.tile([35, A], DT.float32)
            for cc in range(CPC):
                c0 = 32 * cc
                nc.vector.tensor_tensor(sq3[c0:c0 + 3, :], ct2[c0:c0 + 3, :],
                                        ct2[c0:c0 + 3, :], ALU.mult)
                nc.vector.tensor_scalar(m2ct[c0:c0 + 3, :], ct2[c0:c0 + 3, :],
                                        -2.0, None, ALU.mult)
            dsqp = []
            for cc in range(CPC):
                dp = ps.tile([A, A], DT.float32, tag=f"dsq{cc}", name=f"dsq{cc}")
                c0 = 32 * cc
                nc.tensor.matmul(dp[:], sq3[c0:c0 + 3], ones3[c0:c0 + 3],
                                 start=True, stop=False)
                nc.tensor.matmul(dp[:], ones3[c0:c0 + 3], sq3[c0:c0 + 3],
                                 start=False, stop=False)
                nc.tensor.matmul(dp[:], ct2[c0:c0 + 3], m2ct[c0:c0 + 3],
                                 start=False, stop=True)
                dsqp.append(dp)
            for cc in range(CPC):
                nc.vector.tensor_scalar(sqin[A * cc:A * (cc + 1), 0:A], dsqp[cc][:],
                                        1e-12, None, ALU.max)

            # ---------- triple stream geometry (vector-only chain -> Ln) ----
            v12 = sb.tile([A2, NCH, 2, 3], DT.float32)
            nc.vector.tensor_tensor(
                v12[:].rearrange("p c u x -> p c (u x)"),
                rjk[:, :, 0:6], rjk[:, :, 6:12], ALU.subtract)
            sq6 = sb.tile([A2, NCH, 2, 3], DT.float32)
            nc.vector.tensor_tensor(sq6[:], v12[:], v12[:], ALU.mult)
            dq2 = sqin[:, A:A + 2 * NCH].rearrange("p (c u) -> p c u", u=2)
            nc.vector.tensor_reduce(dq2, sq6[:], mybir.AxisListType.X, ALU.add)
            dm3 = sb.tile([A2, NCH, 3], DT.float32)
            nc.vector.tensor_tensor(dm3[:], v12[:, :, 0, :], v12[:, :, 1, :], ALU.mult)
            dot = sb.tile([A2, NCH, 1], DT.float32)
            nc.vector.tensor_reduce(dot[:], dm3[:], mybir.AxisListType.X, ALU.add)
            dqp = sb.tile([A2, NCH, 1], DT.float32)
            nc.vector.tensor_tensor(dqp[:], dq2[:, :, 0:1], dq2[:, :, 1:2], ALU.mult)
            dot2 = sb.tile([A2, NCH, 1], DT.float32)
            nc.vector.tensor_tensor(dot2[:], dot[:], dot[:], ALU.mult)
            rcpq = sb.tile([A2, NCH, 1], DT.float32)
            nc.vector.reciprocal(rcpq[:], dqp[:])
            usq = sb.tile([A2, NCH, 1], DT.float32)
            nc.vector.tensor_tensor(usq[:], dot2[:], rcpq[:], ALU.mult)
            nc.vector.tensor_scalar(
                sqin[:, A + 2 * NCH:NSQ].rearrange("p (c u) -> p c u", u=1),
                usq[:], -0.9025, 1.0, ALU.mult, ALU.add)

            # ---------- Ln + Exp: d = exp(0.5*ln(d^2)), ss likewise.
            # Pair part first (feeds the angular chain), radial part second.
            lnall = sb.tile([A2, NSQ], DT.float32)
            dall = sb.tile([A2, NSQ], DT.float32)
            nc.scalar.activation(lnall[:, A:NSQ], sqin[:, A:NSQ], AF.Ln)
            nc.scalar.activation(dall[:, A:NSQ], lnall[:, A:NSQ], AF.Exp, scale=0.5)
            nc.scalar.activation(lnall[:, 0:A], sqin[:, 0:A], AF.Ln)
            nc.scalar.activation(dall[:, 0:A], lnall[:, 0:A], AF.Exp, scale=0.5)
            d_t = dall[:, 0:A]
            d2l = dall[:, A:A + 2 * NCH].rearrange("p (c u) -> p c u", u=2)
            ss = dall[:, A + 2 * NCH:NSQ].rearrange("p (c u) -> p c u", u=1)

            # ---------- cutoff polynomials in d^2 (vector; overlap Ln/Exp) ----
            msk2 = sb.tile([A2, NCH, 2], DT.float32)
            d2p = sqin[:, A:A + 2 * NCH].rearrange("p (c u) -> p c u", u=2)
            nc.vector.tensor_scalar(msk2[:], d2p, RCA * RCA, None, ALU.is_le)
            g2 = sb.tile([A2, NCH, 2], DT.float32)
            tmpA = sb.tile([A2, NCH, 2], DT.float32)
            _poly5(nc, nc.vector, g2[:], d2p, PC_A, msk2[:], tmpA[:])
            g = sb.tile([A2, NCH, 1], DT.float32)
            nc.gpsimd.tensor_tensor(g[:], g2[:, :, 0:1], g2[:, :, 1:2], ALU.mult)

            mskR = sb.tile([A2, A], DT.float32)
            nc.vector.tensor_scalar(mskR[:], sqin[:, 0:A], RCR * RCR, None, ALU.is_le)
            fcR = sb.tile([A2, A], DT.float32)
            tmpR = sb.tile([A2, A], DT.float32)
            _poly5(nc, nc.vector, fcR[:], sqin[:, 0:A], PC_R, mskR[:], tmpR[:])

            # ---------- post-d angular chain ----------
            # dcz = dot*cos-lane and hs = ss*sin-lane are off the critical
            # path (ready before rcp); hc = dcz*rcp keeps the f1 chain to
            # TWO vector ops after the reciprocal, so Ln/Exp32 win the
            # scalar engine before the radial squares become ready.
            dcz = sb.tile([A2, NCH, 8], DT.float32)
            nc.vector.tensor_tensor(dcz[:], dot[:].broadcast_to([A2, NCH, 8]),
                                    lane(_LN_CZ, 8, NCH), ALU.mult)
            hs = sb.tile([A2, NCH, 8], DT.float32)
            nc.vector.tensor_tensor(hs[:], ss.broadcast_to([A2, NCH, 8]),
                                    lane(_LN_SZ, 8, NCH), ALU.mult)
            prod = sb.tile([A2, NCH, 1], DT.float32)
            nc.gpsimd.tensor_tensor(prod[:], d2l[:, :, 0:1], d2l[:, :, 1:2], ALU.mult)
            tsum = sb.tile([A2, NCH, 1], DT.float32)
            nc.gpsimd.tensor_tensor(tsum[:], d2l[:, :, 0:1], d2l[:, :, 1:2], ALU.add)
            rcp = sb.tile([A2, NCH, 1], DT.float32)
            nc.vector.reciprocal(rcp[:], prod[:])

            hc = sb.tile([A2, NCH, 8], DT.float32)
            nc.vector.tensor_tensor(hc[:], dcz[:],
                                    rcp[:].broadcast_to([A2, NCH, 8]), ALU.mult)
            hcl = sb.tile([A2, NCH, 8], DT.float32)
            nc.vector.tensor_tensor(hcl[:], hc[:], hs[:], ALU.add)
            lnh = sb.tile([A2, NCH, 8], DT.float32)
            f1 = sb.tile([A2, NCH, 8], DT.bfloat16)
            nc.scalar.activation(lnh[:], hcl[:], AF.Ln,
                                 bias=lanes_sb[:, _LN_HALF:_LN_HALF + 1])
            nc.scalar.activation(f1[:], lnh[:], AF.Exp, scale=32.0,
                                 bias=lanes_sb[:, _LN_LN2:_LN_LN2 + 1])

            am = sb.tile([A2, NCH, 4], DT.float32)
            nc.vector.scalar_tensor_tensor(
                am[:], tsum[:].broadcast_to([A2, NCH, 4]), float(np.sqrt(2.0)),
                lane(_LN_SHFA2, 4, NCH), ALU.mult, ALU.subtract)
            amsq = sb.tile([A2, NCH, 4], DT.float32)
            nc.gpsimd.tensor_tensor(amsq[:], am[:], am[:], ALU.mult)
            f2 = sb.tile([A2, NCH, 4], DT.float32)
            nc.scalar.activation(f2[:], amsq[:], AF.Exp, scale=-1.0)

            # radial argument in quarters (fills vector gaps without long
            # displacement of the chain ops)
            rsub = sb.tile([A2, 16, A], DT.float32)
            d_tb = d_t.rearrange("p (r i) -> p r i", r=1)
            for qq in range(4):
                nc.vector.scalar_tensor_tensor(
                    rsub[:, 4 * qq:4 * qq + 4], d_tb.broadcast_to([A2, 4, A]),
                    4.0, shfrq[:, 4 * qq:4 * qq + 4], ALU.mult, ALU.subtract)

            f2g = sb.tile([A2, NCH, 4], DT.bfloat16)
            nc.vector.tensor_tensor(f2g[:], f2[:], g[:].broadcast_to([A2, NCH, 4]),
                                    ALU.mult)
            at = sb.tile([A2, NCH, 32], DT.bfloat16)
            nc.vector.tensor_tensor(
                at[:].rearrange("p c (a z) -> p c a z", a=4),
                f1[:].rearrange("p c (a z) -> p c a z", a=1
                                ).broadcast_to([A2, NCH, 4, 8]),
                f2g[:].rearrange("p c (a z) -> p c a z", z=1
                                 ).broadcast_to([A2, NCH, 4, 8]),
                ALU.mult)

            # species-pair expansion with host-materialized one-hot, all on
            # vector (concurrent gpsimd wide ops just steal DVE bandwidth);
            # 4 slices so the matmuls chase, radial rtf interleaved
            at320 = sb.tile([A2, NCH, NPAIR_T, 32], DT.bfloat16)
            angp = ps.tile([A2, NPAIR_T * 32], DT.float32, tag="angp")

            def build320(c0, c1):
                nc.vector.tensor_tensor(
                    at320[:, c0:c1],
                    at[:, c0:c1].rearrange("p c (s w) -> p c s w", s=1
                                           ).broadcast_to([A2, c1 - c0, NPAIR_T, 32]),
                    ohsp320[:, c0:c1].rearrange("p c (s w) -> p c s w", s=NPAIR_T),
                    ALU.mult)

            # radial: squares + exps on scalar halves, fc multiply on vector
            # interleaved with the build320 slices; radsb keeps the PSUM
            # [species, shift, atom] layout (host permutes)
            rsq = sb.tile([A2, 16, A], DT.float32)
            rte = sb.tile([A2, 16, A], DT.float32)
            rtf = sb.tile([A2, 16, A], DT.bfloat16)
            fcRb = fcR[:].rearrange("p (r i) -> p r i", r=1)
            radsb = sb.tile([2 * NSP, 16, A], DT.float32)
            for half in range(2):
                h0 = 8 * half
                nc.scalar.square(rsq[:, h0:h0 + 8], rsub[:, h0:h0 + 8])
                nc.scalar.activation(rte[:, h0:h0 + 8], rsq[:, h0:h0 + 8],
                                     AF.Exp, scale=-1.0)

            q4 = max(1, NCH // 4)
            cuts = [0, q4, 2 * q4, 3 * q4, NCH]
            radp = []
            for s in range(4):
                if cuts[s + 1] > cuts[s]:
                    build320(cuts[s], cuts[s + 1])
                for ch in range(cuts[s], cuts[s + 1]):
                    nc.tensor.matmul(angp[:], ohci[:, ch, :], at320[:, ch],
                                     start=(ch == 0), stop=(ch == NCH - 1))
                if s in (1, 2):
                    half = s - 1
                    h0 = 8 * half
                    nc.vector.tensor_tensor(rtf[:, h0:h0 + 8], rte[:, h0:h0 + 8],
                                            fcRb.broadcast_to([A2, 8, A]), ALU.mult)
                    rp = ps.tile([2 * NSP, 8, A], DT.float32, tag="radp",
                                 name=f"radp{half}")
                    nc.tensor.matmul(rp[:], ohs_sb[:], rtf[:, h0:h0 + 8, :],
                                     start=True, stop=True)
                    radp.append(rp)
                    nc.scalar.copy(radsb[:, h0:h0 + 8, :], rp[:])
                    nc.gpsimd.dma_start(out[:, h0:h0 + 8, :],
                                        radsb[:, h0:h0 + 8, :])

            # angular: PSUM -> SBUF (split; gpsimd can't read PSUM), DMA
            # halves pipelined behind the copies
            angsb = sb.tile([A2, NPAIR_T * 32], DT.float32)
            nc.scalar.copy(angsb[:, 0:112], angp[:, 0:112])
            nc.sync.dma_start(outa[:, 0:112], angsb[:, 0:112])
            nc.vector.tensor_copy(angsb[:, 112:320], angp[:, 112:320])
            nc.sync.dma_start(outa[:, 112:320], angsb[:, 112:320])

    _split_multiwaits(nc)
    return nc


# ---------------- host side ----------------

def _prep(species, coordinates):
    sp = np.clip(np.asarray(species).astype(np.int64), 0, NSP - 1)
    co = np.ascontiguousarray(np.asarray(coordinates), dtype=np.float32)
    d2 = ((co[:, :, None, :].astype(np.float64) - co[:, None, :, :]) ** 2).sum(-1)
    D = np.sqrt(d2)
    for c in range(C):
        np.fill_diagonal(D[c], 1e9)
    near = D < (RCA + 0.02)

    # triples grouped per center (c, i)
    centers = []  # (count, c, i, J, K)
    for c in range(C):
        for i in range(A):
            nz = np.nonzero(near[c, i])[0]
            m = nz.size
            if m >= 2:
                jj, kk = np.triu_indices(m, k=1)
                centers.append((jj.size, c, i, nz[jj], nz[kk]))
    centers.sort(key=lambda t: -t[0])

    # greedy bin-pack: <=128 centers/core, balance triple counts
    loads = [0] * NCORES
    nbins = [0] * NCORES
    assign = [[] for _ in range(NCORES)]
    for ent in centers:
        k = min((kk for kk in range(NCORES) if nbins[kk] < A2),
                key=lambda kk: loads[kk])
        assign[k].append(ent)
        loads[k] += ent[0]
        nbins[k] += 1

    NCH = max(1, int(np.ceil(max(loads) / 128)))
    KT = NCH * 128

    lane_row = np.zeros(_LN_N, np.float64)
    lane_row[_LN_SHFA2:_LN_SHFA2 + 4] = 2.0 * np.sqrt(2.0) * SHF_A
    lane_row[_LN_CZ:_LN_CZ + 8] = 0.475 * np.cos(SHF_Z)
    lane_row[_LN_SZ:_LN_SZ + 8] = 0.5 * np.sin(SHF_Z)
    lane_row[_LN_LN2] = np.log(2.0)
    lane_row[_LN_HALF] = 0.5

    pad_rjk = np.array([0, 0, 0, 0, 0, 0, 60, 0, 0, 0, 60, 0], np.float32)

    per_core = []
    bin_maps = []
    for k in range(NCORES):
        rjk_l, ci_l, spid_l = [], [], []
        bmap = []
        for b, (cnt, c, i, J, K) in enumerate(assign[k]):
            bmap.append((c, i))
            rjk_l.append(np.concatenate(
                [np.repeat(co[c, i][None], cnt, 0),
                 np.repeat(co[c, i][None], cnt, 0),
                 co[c, J], co[c, K]], axis=1))
            ci_l.append(np.full(cnt, b, np.float32))
            spid_l.append(_tbl[sp[c, J], sp[c, K]].astype(np.float32))
        bin_maps.append(bmap)
        T = sum(x.size for x in ci_l)
        rjk_f = np.empty((KT, 12), np.float32)
        rjk_f[:] = pad_rjk
        ci_f = np.zeros(KT, np.float32)
        spid_f = np.zeros(KT, np.float32)
        if T:
            rjk_f[:T] = np.concatenate(rjk_l, axis=0)
            ci_f[:T] = np.concatenate(ci_l)
            spid_f[:T] = np.concatenate(spid_l)
        # triple t = ch*128 + p  ->  tile [p, ch]
        rjk_t = rjk_f.reshape(NCH, 128, 12).transpose(1, 0, 2).reshape(128, -1)
        ci_t = ci_f.reshape(NCH, 128).T
        spid_t = spid_f.reshape(NCH, 128).T
        lanes_t = np.tile(lane_row.astype(np.float32), (A2, 1))
        combo = np.concatenate([rjk_t, lanes_t], axis=1)

        # host-built one-hots (bf16 exact for 0/1); pair one-hot expanded
        # to the full 320 columns per chunk
        ohci = (ci_t[:, :, None] == np.arange(128, dtype=np.float32)
                ).astype(ml_dtypes.bfloat16).reshape(128, -1)
        ohsp320 = np.repeat(
            (spid_t[:, :, None] == np.arange(NPAIR_T, dtype=np.float32)),
            32, axis=2).astype(ml_dtypes.bfloat16).reshape(128, -1)

        ca, cb = 2 * k, 2 * k + 1
        ohsv = np.zeros((A2, 2 * NSP), np.float32)
        for cc, c in enumerate((ca, cb)):
            ohsv[A * cc:A * (cc + 1), NSP * cc:NSP * (cc + 1)] = (
                sp[c][:, None] == np.arange(NSP))
        ohb = np.concatenate([ohsv.astype(ml_dtypes.bfloat16), ohsp320], axis=1)
        cpad = np.zeros((35, A), np.float32)
        cpad[0:3] = co[ca].T
        cpad[32:35] = co[cb].T
        per_core.append({
            "coords": cpad,
            "combo": np.ascontiguousarray(combo),
            "ohb": np.ascontiguousarray(ohb),
            "ohci": np.ascontiguousarray(ohci),
        })
    return NCH, per_core, bin_maps


def _run(species, coordinates, trace=False):
    NCH, in_maps, bin_maps = _prep(species, coordinates)
    if NCH not in _NC_CACHE:
        _NC_CACHE[NCH] = _build(NCH)
    nc = _NC_CACHE[NCH]
    res = run_bass_kernel_spmd(nc, in_maps, core_ids=list(range(NCORES)), trace=trace)
    full = np.zeros((C, A, 384), np.float32)
    for k in range(NCORES):
        orad = np.asarray(res.results[k]["out"])   # (2*NSP, 16, A)
        oang = np.asarray(res.results[k]["outa"])
        for cc in range(CPC):
            full[2 * k + cc, :, 0:64] = (
                orad[NSP * cc:NSP * (cc + 1)].transpose(2, 0, 1).reshape(A, 64))
        for b, (c, i) in enumerate(bin_maps[k]):
            full[c, i, 64:] = oang[b]
    return full, res


def kernel(species, coordinates):
    out, _ = _run(species, coordinates, trace=False)
    return out
